# revision 1
# baseline (speedup 1.0000x reference)
"""Causal self-attention (B=2, T=2048, D=1024, H=16, rope) on 8 Trainium2 cores.

Sharding: heads are split across cores (2 heads/core, tensor-parallel):
each core computes QKV projection columns for its heads, RoPE, causal
attention, and a partial out-projection (its rows of w_out). The host sums
the 8 partial outputs (the tensor-parallel all-reduce, done at gather time).

All matmul operands are fp16 (fp32 PSUM accumulation). Activations flow
feature-major (transposed) so every matmul contracts along the partition
dim; the host transposes x on the way in and the output back on the way
out. Softmax denominators come free from a ones-column appended to V;
exp runs biased (exp(s/8 - 4)) to stay inside fp16 range, the bias cancels
in the normalization.
"""

import sys

for _p in ("/opt/trn_rl_repo",):
    if _p not in sys.path:
        sys.path.insert(0, _p)

import numpy as np

B, T, D, H = 2, 2048, 1024, 16
DH = D // H  # 64
N_CORES = 8
HPC = H // N_CORES  # heads per core = 2
BT = B * T  # 4096
ROPE_BASE = 10000.0
EXP_BIAS = -4.0

_CACHE = {}


def _host_consts():
    # RoPE tables, feature-major, two heads stacked: [128, T]
    inv_freq = 1.0 / (ROPE_BASE ** (np.arange(0, DH, 2, dtype=np.float32) / DH))
    t = np.arange(T, dtype=np.float32)
    freqs = np.outer(t, inv_freq)  # [T, 32]
    emb = np.concatenate([freqs, freqs], axis=-1)  # [T, 64]
    cosT = np.cos(emb).T.astype(np.float32)  # [64, T]
    sinT = np.sin(emb).T.astype(np.float32)
    # sign baked for the rotate-half term: rows 0:32 get -sin, rows 32:64 +sin
    sinS = np.concatenate([-sinT[:32], sinT[32:]], axis=0)
    cosb = np.concatenate([cosT, cosT], axis=0).astype(np.float16)
    sinb = np.concatenate([sinS, sinS], axis=0).astype(np.float16)
    # Causal masks for the 4 diagonal-block offsets o = 0,128,256,384,
    # concatenated along free dim: [128, 2048]
    p = np.arange(128)[:, None]
    f = np.arange(512)[None, :]
    mask = np.zeros((128, 4 * 512), dtype=np.float16)
    for tno in range(4):
        o = 128 * tno
        mask[:, tno * 512:(tno + 1) * 512] = (f >= o + p).astype(np.float16)
    return cosb, sinb, mask


def _build(debug=False):
    """Build + schedule the per-core Bass module (same program on all cores)."""
    from concourse import bacc
    import concourse.mybir as mybir
    import concourse.tile as tile

    F16 = mybir.dt.float16
    F32 = mybir.dt.float32
    AF = mybir.ActivationFunctionType

    nc = bacc.Bacc("TRN2", target_bir_lowering=False, debug=False,
                   num_devices=N_CORES)

    xt_d = nc.dram_tensor("xt", [D, BT], F16, kind="ExternalInput")
    wq_d = nc.dram_tensor("wq", [D, 128], F16, kind="ExternalInput")
    wk_d = nc.dram_tensor("wk", [D, 128], F16, kind="ExternalInput")
    wv_d = nc.dram_tensor("wv", [D, 128], F16, kind="ExternalInput")
    wo_d = nc.dram_tensor("wo", [128, D], F16, kind="ExternalInput")
    cos_d = nc.dram_tensor("cosb", [128, T], F16, kind="ExternalInput")
    sin_d = nc.dram_tensor("sinb", [128, T], F16, kind="ExternalInput")
    mask_d = nc.dram_tensor("mask", [128, 2048], F16, kind="ExternalInput")
    out_d = nc.dram_tensor("outp", [D, BT], F32, kind="ExternalOutput")
    if debug:
        qt_o = nc.dram_tensor("qt_o", [128, BT], F16, kind="ExternalOutput")
        kt_o = nc.dram_tensor("kt_o", [128, BT], F16, kind="ExternalOutput")
        vt_o = nc.dram_tensor("vt_o", [128, BT], F16, kind="ExternalOutput")
        on_o = nc.dram_tensor("on_o", [128, BT], F16, kind="ExternalOutput")
        vp_o = nc.dram_tensor("vp_o", [128, 16 * 130], F16, kind="ExternalOutput")

    NK = D // 128       # 8 contraction chunks for qkv projection
    NS = BT // 512      # 8 token slices
    NJ = T // 512       # 4 tq slices per batch
    NB = T // 128       # 16 tk blocks per batch
    PIPE = 2            # exp->AV software pipeline depth, in 2-block groups

    with tile.TileContext(nc) as tc:
        with (
            tc.tile_pool(name="consts", bufs=1) as consts,
            tc.tile_pool(name="acts", bufs=1) as acts,
        ):
            wq = consts.tile([128, NK, 128], F16)
            wk = consts.tile([128, NK, 128], F16)
            wv = consts.tile([128, NK, 128], F16)
            wo = consts.tile([128, NK, 128], F16)
            nc.sync.dma_start(out=wq, in_=wq_d[:, :].rearrange("(k p) f -> p k f", p=128))
            nc.sync.dma_start(out=wk, in_=wk_d[:, :].rearrange("(k p) f -> p k f", p=128))
            nc.sync.dma_start(out=wv, in_=wv_d[:, :].rearrange("(k p) f -> p k f", p=128))
            nc.sync.dma_start(out=wo, in_=wo_d[:, :].rearrange("p (m f) -> p m f", m=NK))
            cosb = consts.tile([128, T], F16)
            sinb = consts.tile([128, T], F16)
            mask = consts.tile([128, 2048], F16)
            nc.sync.dma_start(out=cosb, in_=cos_d[:, :])
            nc.sync.dma_start(out=sinb, in_=sin_d[:, :])
            nc.sync.dma_start(out=mask, in_=mask_d[:, :])
            ones16 = consts.tile([128, NB], F16)
            nc.vector.memset(ones16, 1.0)
            ebias = consts.tile([128, 1], F32)
            nc.vector.memset(ebias, EXP_BIAS)

            qt = acts.tile([128, BT], F16)  # rows: [h0 d0..63 | h1 d0..63]
            kt = acts.tile([128, BT], F16)
            vt = acts.tile([128, BT], F16)

            # ---------------- Phase 1: QKV^T projection + RoPE ----------
            with (
                tc.tile_pool(name="xt", bufs=2) as xtp,
                tc.tile_pool(name="rope", bufs=2) as rope,
                tc.tile_pool(name="qkv_ps", bufs=6, space="PSUM") as qkv_ps,
            ):
                xt_r = xt_d[:, :].rearrange("(k p) t -> p k t", p=128)
                for n in range(NS):
                    ts = slice(n * 512, (n + 1) * 512)
                    cs = slice((n % NJ) * 512, (n % NJ) * 512 + 512)
                    xtt = xtp.tile([128, NK, 512], F16, tag="xt")
                    nc.sync.dma_start(out=xtt, in_=xt_r[:, :, ts])
                    pss = []
                    for w in (wq, wk, wv):
                        ps = qkv_ps.tile([128, 512], F32, tag="qkv")
                        for k in range(NK):
                            nc.tensor.matmul(ps, w[:, k, :], xtt[:, k, :],
                                             start=(k == 0), stop=(k == NK - 1))
                        pss.append(ps)
                    # v: plain evacuation (fp16 round on write), on ACT
                    nc.scalar.copy(vt[:, ts], pss[2])
                    # q, k: rope
                    for ps, dst in ((pss[0], qt), (pss[1], kt)):
                        raw = rope.tile([128, 512], F16, tag="raw")
                        nc.scalar.copy(raw, ps)
                        swp = rope.tile([128, 512], F16, tag="swp")
                        for a, b2 in ((0, 32), (32, 0), (64, 96), (96, 64)):
                            nc.sync.dma_start(out=swp[a:a + 32, :],
                                              in_=raw[b2:b2 + 32, :])
                        nc.vector.tensor_mul(raw, raw, cosb[:, cs])
                        nc.vector.tensor_mul(swp, swp, sinb[:, cs])
                        nc.vector.tensor_add(dst[:, ts], raw, swp)

            if debug:
                nc.sync.dma_start(out=qt_o[:, :], in_=qt)
                nc.sync.dma_start(out=kt_o[:, :], in_=kt)
                nc.sync.dma_start(out=vt_o[:, :], in_=vt)

            # ------------- Phase 2+3: attention + out-projection --------
            with (
                tc.tile_pool(name="vp", bufs=1) as vpp,
                tc.tile_pool(name="est", bufs=4) as estp,
                tc.tile_pool(name="on", bufs=2) as onp,
                tc.tile_pool(name="inv", bufs=2) as invp,
                tc.tile_pool(name="oev", bufs=4) as oevp,
                tc.tile_pool(name="st_ps", bufs=3, space="PSUM") as st_ps,
                tc.tile_pool(name="u_ps", bufs=2, space="PSUM") as u_ps,
            ):
                for b in range(B):
                    t0 = b * T
                    # V' = [V_h | 1] token-major via DMA transpose; inner
                    # stride 80 elems = 160B keeps every transpose dst
                    # 32B-aligned (unaligned dsts corrupt silently)
                    vph = [vpp.tile([128, NB, 80], F16, tag=f"vp{h}",
                                    name=f"vp{h}_{b}")
                           for h in range(HPC)]
                    for i in range(NB):
                        blk = slice(t0 + i * 128, t0 + (i + 1) * 128)
                        for h in range(HPC):
                            nc.sync.dma_start_transpose(
                                out=vph[h][:, i, 0:64],
                                in_=vt[h * 64:(h + 1) * 64, blk])
                    for h in range(HPC):
                        nc.vector.tensor_copy(vph[h][:, :, 64], ones16)

                    on = onp.tile([128, T], F16, tag="on")
                    for h in range(HPC):
                        hp = h * 64
                        vp = vph[h]
                        for j in range(NJ):
                            qs = slice(t0 + j * 512, t0 + (j + 1) * 512)
                            nblk = 4 * j + 4
                            ngrp = nblk // 2
                            u = u_ps.tile([65, 512], F32, tag="u")

                            def do_av(est0, blks):
                                for t2, i in enumerate(blks):
                                    nc.tensor.matmul(
                                        u, vp[:, i, 0:65],
                                        est0[:, t2 * 512:(t2 + 1) * 512],
                                        start=(i == 0), stop=(i == nblk - 1))

                            pend = []
                            for g in range(ngrp):
                                st = st_ps.tile([128, 1024], F32, tag="st")
                                for t2 in range(2):
                                    i = 2 * g + t2
                                    nc.tensor.matmul(
                                        st[:, t2 * 512:(t2 + 1) * 512],
                                        kt[hp:hp + 64,
                                           t0 + i * 128: t0 + (i + 1) * 128],
                                        qt[hp:hp + 64, qs],
                                        start=True, stop=True)
                                est = estp.tile([128, 1024], F16, tag="est")
                                nc.scalar.activation(est, st, AF.Exp,
                                                     scale=float(DH) ** -0.5,
                                                     bias=ebias)
                                if 2 * g >= 4 * j:  # diagonal group
                                    mo = (2 * g - 4 * j) * 512
                                    nc.vector.tensor_mul(
                                        est, est, mask[:, mo:mo + 1024])
                                pend.append((est, (2 * g, 2 * g + 1)))
                                if len(pend) > PIPE:
                                    do_av(*pend.pop(0))
                            while pend:
                                do_av(*pend.pop(0))

                            # normalize: bcast r (gpsimd), approx 1/r on a
                            # full tile (custom DVE op mishandles sliced
                            # single-partition inputs), multiply on DVE
                            rrow = invp.tile([1, 512], F32, tag="rrow")
                            nc.vector.tensor_copy(rrow, u[64:65, :])
                            bc = invp.tile([64, 512], F32, tag="bc")
                            nc.gpsimd.partition_broadcast(bc, rrow)
                            bci = invp.tile([64, 512], F32, tag="bci")
                            nc.vector.reciprocal_approx_fast(bci, bc)
                            nc.vector.tensor_mul(
                                on[hp:hp + 64, j * 512:(j + 1) * 512],
                                u[0:64, :], bci)

                    if debug:
                        nc.sync.dma_start(out=on_o[:, t0:t0 + T], in_=on)
                        if b == 0:
                            nc.sync.dma_start(
                                out=vp_o[:, :].rearrange("p (a c) -> p a c",
                                                         a=16),
                                in_=vp[:, :, :])
                    # out-projection for batch b: wo.T @ on; pairs of dout
                    # chunks share one [128,1024] psum tile from the st pool
                    for j in range(NJ):
                        ons = on[:, j * 512:(j + 1) * 512]
                        for m2 in range(NK // 2):
                            op = st_ps.tile([128, 1024], F32, tag="st")
                            for t2 in range(2):
                                m = 2 * m2 + t2
                                nc.tensor.matmul(
                                    op[:, t2 * 512:(t2 + 1) * 512],
                                    wo[:, m, :], ons, start=True, stop=True)
                            ot = oevp.tile([128, 1024], F32, tag="ot")
                            if m2 % 2 == 0:
                                nc.vector.tensor_copy(ot, op)
                            else:
                                nc.scalar.copy(ot, op)
                            for t2 in range(2):
                                m = 2 * m2 + t2
                                nc.sync.dma_start(
                                    out=out_d[m * 128:(m + 1) * 128,
                                              t0 + j * 512: t0 + (j + 1) * 512],
                                    in_=ot[:, t2 * 512:(t2 + 1) * 512])

    nc.compile()
    return nc


def _get_nc(debug=False):
    key = "ncd" if debug else "nc"
    if key not in _CACHE:
        _CACHE[key] = _build(debug)
    return _CACHE[key]


def _run(nc, in_maps, trace=False):
    from concourse.bass_utils import run_bass_kernel_spmd

    last = None
    for attempt in range(3):
        try:
            return run_bass_kernel_spmd(nc, in_maps,
                                        core_ids=list(range(N_CORES)),
                                        trace=trace)
        except Exception as e:  # transient device faults: retry
            last = e
            if "UNRECOVERABLE" not in str(e) and "UNAVAILABLE" not in str(e):
                raise
    raise last


def kernel(x, w_qkv, w_out, _trace=False, _debug=False):
    x = np.asarray(x, dtype=np.float32)
    w_qkv = np.asarray(w_qkv, dtype=np.float32)
    w_out = np.asarray(w_out, dtype=np.float32)

    xt = np.ascontiguousarray(x.reshape(BT, D).T).astype(np.float16)
    cosb, sinb, mask = _host_consts()

    in_maps = []
    for c in range(N_CORES):
        h0 = HPC * c
        cols = np.arange(h0 * DH, (h0 + HPC) * DH)
        in_maps.append({
            "xt": xt,
            "wq": np.ascontiguousarray(w_qkv[:, cols]).astype(np.float16),
            "wk": np.ascontiguousarray(w_qkv[:, D + cols]).astype(np.float16),
            "wv": np.ascontiguousarray(w_qkv[:, 2 * D + cols]).astype(np.float16),
            "wo": np.ascontiguousarray(w_out[cols, :]).astype(np.float16),
            "cosb": cosb,
            "sinb": sinb,
            "mask": mask,
        })

    nc = _get_nc(_debug)
    res = _run(nc, in_maps, trace=_trace)
    acc = np.zeros((D, BT), dtype=np.float64)
    for c in range(N_CORES):
        acc += res.results[c]["outp"]
    out = acc.T.astype(np.float32).reshape(B, T, D)
    if _debug:
        return out, res
    if _trace:
        return out, res
    return out



# revision 2
# speedup vs baseline: 1.2312x; 1.2312x over previous
"""Causal self-attention (B=2, T=2048, D=1024, H=16, rope) on 8 Trainium2 cores.

Sharding: heads are split across cores (2 heads/core, tensor-parallel):
each core computes QKV projection columns for its heads, RoPE, causal
attention, and a partial out-projection (its rows of w_out). The host sums
the 8 partial outputs (the tensor-parallel all-reduce, done at gather time).

All matmul operands are fp16 (fp32 PSUM accumulation). Activations flow
feature-major (transposed) so every matmul contracts along the partition
dim; the host transposes x on the way in and the output back on the way
out. Softmax denominators come free from a ones-column appended to V;
exp runs biased (exp(s/8 - 4)) to stay inside fp16 range, the bias cancels
in the normalization.

Perf structure: the two heads' score matmuls are issued back-to-back so the
PE runs them concurrently in disjoint 64-row groups (K=64 each). V is
transposed on the tensor engine (transpose-mode matmul against identity)
instead of DMA-transpose, keeping the Sync engine free for real DMAs. The
out-projection stages fp16 results in SBUF and ships one DMA per (batch,
512-token chunk). PE work is emitted in one continuous stream (QKV ->
transposes -> attention -> out-proj interleaved across batches) to keep
the PE clock un-throttled.
"""

import sys

for _p in ("/opt/trn_rl_repo",):
    if _p not in sys.path:
        sys.path.insert(0, _p)

import numpy as np

B, T, D, H = 2, 2048, 1024, 16
DH = D // H  # 64
N_CORES = 8
HPC = H // N_CORES  # heads per core = 2
BT = B * T  # 4096
ROPE_BASE = 10000.0
EXP_BIAS = -4.0

_CACHE = {}


def _host_consts():
    # RoPE tables, feature-major, two heads stacked: [128, T]
    inv_freq = 1.0 / (ROPE_BASE ** (np.arange(0, DH, 2, dtype=np.float32) / DH))
    t = np.arange(T, dtype=np.float32)
    freqs = np.outer(t, inv_freq)  # [T, 32]
    emb = np.concatenate([freqs, freqs], axis=-1)  # [T, 64]
    cosT = np.cos(emb).T.astype(np.float32)  # [64, T]
    sinT = np.sin(emb).T.astype(np.float32)
    # sign baked for the rotate-half term: rows 0:32 get -sin, rows 32:64 +sin
    sinS = np.concatenate([-sinT[:32], sinT[32:]], axis=0)
    cosb = np.concatenate([cosT, cosT], axis=0).astype(np.float16)
    sinb = np.concatenate([sinS, sinS], axis=0).astype(np.float16)
    # Causal masks for the 4 diagonal-block offsets o = 0,128,256,384,
    # concatenated along free dim: [128, 2048]
    p = np.arange(128)[:, None]
    f = np.arange(512)[None, :]
    mask = np.zeros((128, 4 * 512), dtype=np.float16)
    for tno in range(4):
        o = 128 * tno
        mask[:, tno * 512:(tno + 1) * 512] = (f >= o + p).astype(np.float16)
    ident = np.eye(128, dtype=np.float16)
    return cosb, sinb, mask, ident


def _build():
    """Build + schedule the per-core Bass module (same program on all cores)."""
    from concourse import bacc
    import concourse.mybir as mybir
    import concourse.tile as tile

    F16 = mybir.dt.float16
    F32 = mybir.dt.float32
    AF = mybir.ActivationFunctionType

    nc = bacc.Bacc("TRN2", target_bir_lowering=False, debug=False,
                   num_devices=N_CORES)

    xt_d = nc.dram_tensor("xt", [D, BT], F16, kind="ExternalInput")
    wq_d = nc.dram_tensor("wq", [D, 128], F16, kind="ExternalInput")
    wk_d = nc.dram_tensor("wk", [D, 128], F16, kind="ExternalInput")
    wv_d = nc.dram_tensor("wv", [D, 128], F16, kind="ExternalInput")
    wo_d = nc.dram_tensor("wo", [128, D], F16, kind="ExternalInput")
    cos_d = nc.dram_tensor("cosb", [128, T], F16, kind="ExternalInput")
    sin_d = nc.dram_tensor("sinb", [128, T], F16, kind="ExternalInput")
    mask_d = nc.dram_tensor("mask", [128, 2048], F16, kind="ExternalInput")
    id_d = nc.dram_tensor("ident", [128, 128], F16, kind="ExternalInput")
    out_d = nc.dram_tensor("outp", [D, BT], F16, kind="ExternalOutput")

    NK = D // 128       # 8 contraction chunks for qkv projection
    NS = BT // 512      # 8 token slices
    NJ = T // 512       # 4 tq slices per batch
    NB = T // 128       # 16 tk blocks per batch
    PIPE = 2            # exp->AV software pipeline depth, in 2-block groups

    with tile.TileContext(nc) as tc:
        with (
            tc.tile_pool(name="consts", bufs=1) as consts,
            tc.tile_pool(name="acts", bufs=1) as acts,
        ):
            wq = consts.tile([128, NK, 128], F16)
            wk = consts.tile([128, NK, 128], F16)
            wv = consts.tile([128, NK, 128], F16)
            wo = consts.tile([128, NK, 128], F16)
            nc.sync.dma_start(out=wq, in_=wq_d[:, :].rearrange("(k p) f -> p k f", p=128))
            nc.sync.dma_start(out=wk, in_=wk_d[:, :].rearrange("(k p) f -> p k f", p=128))
            nc.sync.dma_start(out=wv, in_=wv_d[:, :].rearrange("(k p) f -> p k f", p=128))
            nc.sync.dma_start(out=wo, in_=wo_d[:, :].rearrange("p (m f) -> p m f", m=NK))
            cosb = consts.tile([128, T], F16)
            sinb = consts.tile([128, T], F16)
            mask = consts.tile([128, 2048], F16)
            idt = consts.tile([128, 128], F16)
            nc.sync.dma_start(out=cosb, in_=cos_d[:, :])
            nc.sync.dma_start(out=sinb, in_=sin_d[:, :])
            nc.sync.dma_start(out=mask, in_=mask_d[:, :])
            nc.sync.dma_start(out=idt, in_=id_d[:, :])
            ebias = consts.tile([128, 1], F32)
            nc.vector.memset(ebias, EXP_BIAS)

            qt = acts.tile([128, BT], F16)  # rows: [h0 d0..63 | h1 d0..63]
            kt = acts.tile([128, BT], F16)
            vt = acts.tile([128, BT], F16)

            # ---------------- Phase 1: QKV^T projection + RoPE ----------
            with (
                tc.tile_pool(name="xt", bufs=2) as xtp,
                tc.tile_pool(name="rope", bufs=2) as rope,
                tc.tile_pool(name="qkv_ps", bufs=6, space="PSUM") as qkv_ps,
            ):
                xt_r = xt_d[:, :].rearrange("(k p) t -> p k t", p=128)
                for n in range(NS):
                    ts = slice(n * 512, (n + 1) * 512)
                    cs = slice((n % NJ) * 512, (n % NJ) * 512 + 512)
                    xtt = xtp.tile([128, NK, 512], F16, tag="xt")
                    nc.sync.dma_start(out=xtt, in_=xt_r[:, :, ts])
                    pss = []
                    for w in (wq, wk, wv):
                        ps = qkv_ps.tile([128, 512], F32, tag="qkv")
                        for k in range(NK):
                            nc.tensor.matmul(ps, w[:, k, :], xtt[:, k, :],
                                             start=(k == 0), stop=(k == NK - 1))
                        pss.append(ps)
                    # v: plain evacuation (fp16 round on write), on ACT
                    nc.scalar.copy(vt[:, ts], pss[2])
                    # q, k: rope
                    for ps, dst in ((pss[0], qt), (pss[1], kt)):
                        raw = rope.tile([128, 512], F16, tag="raw")
                        nc.scalar.copy(raw, ps)
                        swp = rope.tile([128, 512], F16, tag="swp")
                        for a, b2 in ((0, 32), (32, 0), (64, 96), (96, 64)):
                            nc.sync.dma_start(out=swp[a:a + 32, :],
                                              in_=raw[b2:b2 + 32, :])
                        nc.vector.tensor_mul(raw, raw, cosb[:, cs])
                        nc.vector.tensor_mul(swp, swp, sinb[:, cs])
                        nc.vector.tensor_add(dst[:, ts], raw, swp)

            # ------------- Phase 2+3: attention + out-projection --------
            with (
                tc.tile_pool(name="vp", bufs=2) as vpp,
                tc.tile_pool(name="est", bufs=6) as estp,
                tc.tile_pool(name="on", bufs=2) as onp,
                tc.tile_pool(name="inv", bufs=2) as invp,
                tc.tile_pool(name="ot", bufs=2) as otp,
                tc.tile_pool(name="st_ps", bufs=3, space="PSUM") as st_ps,
                tc.tile_pool(name="u_ps", bufs=2, space="PSUM") as u_ps,
            ):
                vph = {}  # (h, b) -> [128, NB, 80] token-major V' tiles
                ons = {}  # b -> [128, T] normalized attention output

                def v_transpose(b):
                    """V' = [V_h | 1] token-major via PE transpose-mode."""
                    t0 = b * T
                    for h in range(HPC):
                        vph[(h, b)] = vpp.tile([128, NB, 80], F16,
                                               tag=f"vp{h}", name=f"vp{h}_{b}")
                    for g4 in range(NB // 4):
                        tp = st_ps.tile([128, 4, 128], F16, tag="st",
                                        name=f"tp_{b}_{g4}")
                        for t2 in range(4):
                            i = 4 * g4 + t2
                            blk = slice(t0 + i * 128, t0 + (i + 1) * 128)
                            nc.tensor.transpose(tp[:, t2, :], vt[:, blk], idt)
                        for h in range(HPC):
                            src = tp[:, :, h * 64:(h + 1) * 64]
                            dst = vph[(h, b)][:, 4 * g4:4 * g4 + 4, 0:64]
                            if (g4 + h) % 2 == 0:
                                nc.vector.tensor_copy(dst, src)
                            else:
                                nc.scalar.copy(dst, src)
                    for h in range(HPC):
                        nc.vector.memset(vph[(h, b)][:, :, 64:65], 1.0)

                def attention(b):
                    t0 = b * T
                    on = onp.tile([128, T], F16, tag="on", name=f"on_{b}")
                    ons[b] = on
                    for j in range(NJ):
                        qs = slice(t0 + j * 512, t0 + (j + 1) * 512)
                        nblk = 4 * j + 4
                        ngrp = nblk // 2
                        u = {h: u_ps.tile([65, 512], F32, tag="u",
                                          name=f"u{h}_{b}_{j}")
                             for h in range(HPC)}

                        def do_av(est, blks):
                            for t2, i in enumerate(blks):
                                for h in range(HPC):
                                    nc.tensor.matmul(
                                        u[h], vph[(h, b)][:, i, 0:65],
                                        est[h][:, t2 * 512:(t2 + 1) * 512],
                                        start=(i == 0), stop=(i == nblk - 1))

                        pend = []
                        for g in range(ngrp):
                            st = {h: st_ps.tile([128, 1024], F32, tag="st",
                                                name=f"st{h}_{b}_{j}_{g}")
                                  for h in range(HPC)}
                            # issue the two heads' matmuls back-to-back:
                            # disjoint 64-row groups run concurrently on PE
                            for t2 in range(2):
                                i = 2 * g + t2
                                ks = slice(t0 + i * 128, t0 + (i + 1) * 128)
                                for h in range(HPC):
                                    hp = h * 64
                                    nc.tensor.matmul(
                                        st[h][:, t2 * 512:(t2 + 1) * 512],
                                        kt[hp:hp + 64, ks],
                                        qt[hp:hp + 64, qs],
                                        start=True, stop=True)
                            est = {}
                            for h in range(HPC):
                                est[h] = estp.tile([128, 1024], F16, tag="est",
                                                   name=f"est{h}_{b}_{j}_{g}")
                                nc.scalar.activation(est[h], st[h], AF.Exp,
                                                     scale=float(DH) ** -0.5,
                                                     bias=ebias)
                                if 2 * g >= 4 * j:  # diagonal group
                                    mo = (2 * g - 4 * j) * 512
                                    nc.vector.tensor_mul(
                                        est[h], est[h], mask[:, mo:mo + 1024])
                            pend.append((est, (2 * g, 2 * g + 1)))
                            if len(pend) > PIPE:
                                do_av(*pend.pop(0))
                        while pend:
                            do_av(*pend.pop(0))

                        # normalize: bcast r (gpsimd), approx 1/r on a
                        # full tile, multiply on DVE
                        for h in range(HPC):
                            hp = h * 64
                            rrow = invp.tile([1, 512], F32, tag="rrow",
                                             name=f"rr{h}_{b}_{j}")
                            nc.vector.tensor_copy(rrow, u[h][64:65, :])
                            bc = invp.tile([64, 512], F32, tag="bc",
                                           name=f"bc{h}_{b}_{j}")
                            nc.gpsimd.partition_broadcast(bc, rrow)
                            bci = invp.tile([64, 512], F32, tag="bci",
                                            name=f"bci{h}_{b}_{j}")
                            nc.vector.reciprocal_approx_fast(bci, bc)
                            nc.vector.tensor_mul(
                                on[hp:hp + 64, j * 512:(j + 1) * 512],
                                u[h][0:64, :], bci)

                def outproj(b):
                    t0 = b * T
                    on = ons[b]
                    for j in range(NJ):
                        onj = on[:, j * 512:(j + 1) * 512]
                        ot = otp.tile([128, NK * 512], F16, tag="ot",
                                      name=f"ot_{b}_{j}")
                        for m2 in range(NK // 2):
                            op = st_ps.tile([128, 1024], F32, tag="st",
                                            name=f"op_{b}_{j}_{m2}")
                            for t2 in range(2):
                                m = 2 * m2 + t2
                                nc.tensor.matmul(
                                    op[:, t2 * 512:(t2 + 1) * 512],
                                    wo[:, m, :], onj, start=True, stop=True)
                            dst = ot[:, m2 * 1024:(m2 + 1) * 1024]
                            if m2 % 2 == 0:
                                nc.vector.tensor_copy(dst, op)
                            else:
                                nc.scalar.copy(dst, op)
                        nc.sync.dma_start(
                            out=out_d[:, t0 + j * 512: t0 + (j + 1) * 512]
                            .rearrange("(m p) t -> p m t", p=128),
                            in_=ot)

                v_transpose(0)
                attention(0)
                v_transpose(1)
                outproj(0)
                attention(1)
                outproj(1)

    nc.compile()
    return nc


def _get_nc():
    if "nc" not in _CACHE:
        _CACHE["nc"] = _build()
    return _CACHE["nc"]


def _run(nc, in_maps, trace=False):
    from concourse.bass_utils import run_bass_kernel_spmd

    last = None
    for attempt in range(3):
        try:
            return run_bass_kernel_spmd(nc, in_maps,
                                        core_ids=list(range(N_CORES)),
                                        trace=trace)
        except Exception as e:  # transient device faults: retry
            last = e
            if "UNRECOVERABLE" not in str(e) and "UNAVAILABLE" not in str(e):
                raise
    raise last


def kernel(x, w_qkv, w_out, _trace=False):
    x = np.asarray(x, dtype=np.float32)
    w_qkv = np.asarray(w_qkv, dtype=np.float32)
    w_out = np.asarray(w_out, dtype=np.float32)

    xt = np.ascontiguousarray(x.reshape(BT, D).T).astype(np.float16)
    cosb, sinb, mask, ident = _host_consts()

    in_maps = []
    for c in range(N_CORES):
        h0 = HPC * c
        cols = np.arange(h0 * DH, (h0 + HPC) * DH)
        in_maps.append({
            "xt": xt,
            "wq": np.ascontiguousarray(w_qkv[:, cols]).astype(np.float16),
            "wk": np.ascontiguousarray(w_qkv[:, D + cols]).astype(np.float16),
            "wv": np.ascontiguousarray(w_qkv[:, 2 * D + cols]).astype(np.float16),
            "wo": np.ascontiguousarray(w_out[cols, :]).astype(np.float16),
            "cosb": cosb,
            "sinb": sinb,
            "mask": mask,
            "ident": ident,
        })

    nc = _get_nc()
    res = _run(nc, in_maps, trace=_trace)
    acc = np.zeros((D, BT), dtype=np.float64)
    for c in range(N_CORES):
        acc += res.results[c]["outp"]
    out = acc.T.astype(np.float32).reshape(B, T, D)
    if _trace:
        return out, res
    return out


# revision 3
# speedup vs baseline: 1.2555x; 1.0197x over previous
"""Causal self-attention (B=2, T=2048, D=1024, H=16, rope) on 8 Trainium2 cores.

Sharding: heads are split across cores (2 heads/core, tensor-parallel):
each core computes QKV projection columns for its heads, RoPE, causal
attention, and a partial out-projection (its rows of w_out). The host sums
the 8 partial outputs (the tensor-parallel all-reduce, done at gather time).

All matmul operands are fp16 (fp32 PSUM accumulation). Activations flow
feature-major (transposed) so every matmul contracts along the partition
dim; the host transposes x on the way in and the output back on the way
out. Softmax denominators come free from a ones-column appended to V;
exp runs biased (exp(s/8 - 4)) to stay inside fp16 range, the bias cancels
in the normalization.

Perf structure:
- Host pre-arranges every DRAM tensor so each DMA is one contiguous run
  per partition (multi-KB packets); x^T is loaded once into SBUF and stays
  resident. The output uses a permuted [128, NS, NK, 512] layout that the
  host un-permutes, so stores are contiguous too.
- The two heads' score matmuls are issued back-to-back: the PE runs them
  concurrently in disjoint 64-row groups (K=64 row-tiling).
- V is transposed on the tensor engine (transpose-mode matmul against
  identity) instead of DMA-transpose.
- The attention phase is exp(ACT)-bound, so out-projection chunks and the
  other batch's V-transposes are woven into the attention group loop to
  fill the PE bubbles and keep the PE clock un-throttled.
"""

import sys

for _p in ("/opt/trn_rl_repo",):
    if _p not in sys.path:
        sys.path.insert(0, _p)

import numpy as np

B, T, D, H = 2, 2048, 1024, 16
DH = D // H  # 64
N_CORES = 8
HPC = H // N_CORES  # heads per core = 2
BT = B * T  # 4096
ROPE_BASE = 10000.0
EXP_BIAS = -4.0

NK = D // 128       # 8 contraction chunks for qkv projection
NS = BT // 512      # 8 token slices
NJ = T // 512       # 4 tq slices per batch
NB = T // 128       # 16 tk blocks per batch
PIPE = 2            # exp->AV software pipeline depth, in 2-block groups

_CACHE = {}


def _host_consts():
    # RoPE tables, feature-major, two heads stacked: [128, T]
    inv_freq = 1.0 / (ROPE_BASE ** (np.arange(0, DH, 2, dtype=np.float32) / DH))
    t = np.arange(T, dtype=np.float32)
    freqs = np.outer(t, inv_freq)  # [T, 32]
    emb = np.concatenate([freqs, freqs], axis=-1)  # [T, 64]
    cosT = np.cos(emb).T.astype(np.float32)  # [64, T]
    sinT = np.sin(emb).T.astype(np.float32)
    # sign baked for the rotate-half term: rows 0:32 get -sin, rows 32:64 +sin
    sinS = np.concatenate([-sinT[:32], sinT[32:]], axis=0)
    cosb = np.concatenate([cosT, cosT], axis=0).astype(np.float16)
    sinb = np.concatenate([sinS, sinS], axis=0).astype(np.float16)
    # Causal masks for the 4 diagonal-block offsets o = 0,128,256,384,
    # concatenated along free dim: [128, 2048]
    p = np.arange(128)[:, None]
    f = np.arange(512)[None, :]
    mask = np.zeros((128, 4 * 512), dtype=np.float16)
    for tno in range(4):
        o = 128 * tno
        mask[:, tno * 512:(tno + 1) * 512] = (f >= o + p).astype(np.float16)
    ident = np.eye(128, dtype=np.float16)
    return cosb, sinb, mask, ident


def _prearrange_w(w):
    # [D, 128] -> [128, NK, 128]: partition-contiguous for one-run DMA
    return np.ascontiguousarray(
        w.reshape(NK, 128, 128).transpose(1, 0, 2)).astype(np.float16)


def _build():
    """Build + schedule the per-core Bass module (same program on all cores)."""
    from concourse import bacc
    import concourse.mybir as mybir
    import concourse.tile as tile

    F16 = mybir.dt.float16
    F32 = mybir.dt.float32
    AF = mybir.ActivationFunctionType

    nc = bacc.Bacc("TRN2", target_bir_lowering=False, debug=False,
                   num_devices=N_CORES)

    xt_d = nc.dram_tensor("xt", [128, NS, NK, 512], F16, kind="ExternalInput")
    wq_d = nc.dram_tensor("wq", [128, NK, 128], F16, kind="ExternalInput")
    wk_d = nc.dram_tensor("wk", [128, NK, 128], F16, kind="ExternalInput")
    wv_d = nc.dram_tensor("wv", [128, NK, 128], F16, kind="ExternalInput")
    wo_d = nc.dram_tensor("wo", [128, NK, 128], F16, kind="ExternalInput")
    cos_d = nc.dram_tensor("cosb", [128, T], F16, kind="ExternalInput")
    sin_d = nc.dram_tensor("sinb", [128, T], F16, kind="ExternalInput")
    mask_d = nc.dram_tensor("mask", [128, 2048], F16, kind="ExternalInput")
    id_d = nc.dram_tensor("ident", [128, 128], F16, kind="ExternalInput")
    out_d = nc.dram_tensor("outp", [128, NS, NK, 512], F16,
                           kind="ExternalOutput")

    with tile.TileContext(nc) as tc:
        with (
            tc.tile_pool(name="consts", bufs=1) as consts,
            tc.tile_pool(name="acts", bufs=1) as acts,
        ):
            # weights first (gate the first matmuls), then x^T fully
            # resident, then the late-use constants on the ACT queue
            wq = consts.tile([128, NK, 128], F16)
            wk = consts.tile([128, NK, 128], F16)
            wv = consts.tile([128, NK, 128], F16)
            nc.sync.dma_start(out=wq, in_=wq_d[:, :, :])
            nc.sync.dma_start(out=wk, in_=wk_d[:, :, :])
            nc.sync.dma_start(out=wv, in_=wv_d[:, :, :])
            xt_sb = acts.tile([128, NS, NK, 512], F16)
            for n in range(NS):
                nc.sync.dma_start(out=xt_sb[:, n], in_=xt_d[:, n])
            wo = consts.tile([128, NK, 128], F16)
            cosb = consts.tile([128, T], F16)
            sinb = consts.tile([128, T], F16)
            mask = consts.tile([128, 2048], F16)
            idt = consts.tile([128, 128], F16)
            nc.scalar.dma_start(out=cosb, in_=cos_d[:, :])
            nc.scalar.dma_start(out=sinb, in_=sin_d[:, :])
            nc.scalar.dma_start(out=wo, in_=wo_d[:, :, :])
            nc.scalar.dma_start(out=mask, in_=mask_d[:, :])
            nc.scalar.dma_start(out=idt, in_=id_d[:, :])
            ebias = consts.tile([128, 1], F32)
            nc.vector.memset(ebias, EXP_BIAS)

            qt = acts.tile([128, BT], F16)  # rows: [h0 d0..63 | h1 d0..63]
            kt = acts.tile([128, BT], F16)
            vt = acts.tile([128, BT], F16)

            # ---------------- Phase 1: QKV^T projection + RoPE ----------
            with (
                tc.tile_pool(name="rope", bufs=2) as rope,
                tc.tile_pool(name="qkv_ps", bufs=6, space="PSUM") as qkv_ps,
            ):
                for n in range(NS):
                    ts = slice(n * 512, (n + 1) * 512)
                    cs = slice((n % NJ) * 512, (n % NJ) * 512 + 512)
                    pss = []
                    for w in (wq, wk, wv):
                        ps = qkv_ps.tile([128, 512], F32, tag="qkv")
                        for k in range(NK):
                            nc.tensor.matmul(ps, w[:, k, :], xt_sb[:, n, k, :],
                                             start=(k == 0), stop=(k == NK - 1))
                        pss.append(ps)
                    # v: plain evacuation (fp16 round on write), on ACT
                    nc.scalar.copy(vt[:, ts], pss[2])
                    # q, k: rope
                    for ps, dst in ((pss[0], qt), (pss[1], kt)):
                        raw = rope.tile([128, 512], F16, tag="raw")
                        nc.scalar.copy(raw, ps)
                        swp = rope.tile([128, 512], F16, tag="swp")
                        for a, b2 in ((0, 32), (32, 0), (64, 96), (96, 64)):
                            nc.sync.dma_start(out=swp[a:a + 32, :],
                                              in_=raw[b2:b2 + 32, :])
                        nc.vector.tensor_mul(raw, raw, cosb[:, cs])
                        nc.vector.tensor_mul(swp, swp, sinb[:, cs])
                        nc.vector.tensor_add(dst[:, ts], raw, swp)

            # ------------- Phase 2+3: attention + out-projection --------
            with (
                tc.tile_pool(name="vp", bufs=2) as vpp,
                tc.tile_pool(name="est", bufs=6) as estp,
                tc.tile_pool(name="on", bufs=2) as onp,
                tc.tile_pool(name="inv", bufs=2) as invp,
                tc.tile_pool(name="ot", bufs=2) as otp,
                tc.tile_pool(name="st_ps", bufs=3, space="PSUM") as st_ps,
                tc.tile_pool(name="u_ps", bufs=2, space="PSUM") as u_ps,
            ):
                vph = {}  # (h, b) -> [128, NB, 80] token-major V' tiles
                ons = {}  # b -> [128, T] normalized attention output
                filler = []  # PE work chunks woven into attention bubbles

                def v_alloc(b):
                    for h in range(HPC):
                        vph[(h, b)] = vpp.tile([128, NB, 80], F16,
                                               tag=f"vp{h}", name=f"vp{h}_{b}")

                def v_transpose_chunk(b, g4):
                    """Transpose 4 of batch b's V blocks token-major on PE."""
                    t0 = b * T
                    tp = st_ps.tile([128, 4, 128], F16, tag="st",
                                    name=f"tp_{b}_{g4}")
                    for t2 in range(4):
                        i = 4 * g4 + t2
                        blk = slice(t0 + i * 128, t0 + (i + 1) * 128)
                        nc.tensor.transpose(tp[:, t2, :], vt[:, blk], idt)
                    for h in range(HPC):
                        src = tp[:, :, h * 64:(h + 1) * 64]
                        dst = vph[(h, b)][:, 4 * g4:4 * g4 + 4, 0:64]
                        if (g4 + h) % 2 == 0:
                            nc.vector.tensor_copy(dst, src)
                        else:
                            nc.scalar.copy(dst, src)
                    if g4 == NB // 4 - 1:
                        for h in range(HPC):
                            nc.vector.memset(vph[(h, b)][:, :, 64:65], 1.0)

                def outproj_chunk(b, j):
                    """Partial out-projection for 512 tokens; evac on DVE."""
                    on = ons[b]
                    onj = on[:, j * 512:(j + 1) * 512]
                    ot = otp.tile([128, NK * 512], F16, tag="ot",
                                  name=f"ot_{b}_{j}")
                    for m2 in range(NK // 2):
                        op = st_ps.tile([128, 1024], F32, tag="st",
                                        name=f"op_{b}_{j}_{m2}")
                        for t2 in range(2):
                            m = 2 * m2 + t2
                            nc.tensor.matmul(
                                op[:, t2 * 512:(t2 + 1) * 512],
                                wo[:, m, :], onj, start=True, stop=True)
                        nc.vector.tensor_copy(
                            ot[:, m2 * 1024:(m2 + 1) * 1024], op)
                    nc.sync.dma_start(out=out_d[:, b * NJ + j], in_=ot)

                def attention(b):
                    t0 = b * T
                    on = onp.tile([128, T], F16, tag="on", name=f"on_{b}")
                    ons[b] = on
                    for j in range(NJ):
                        qs = slice(t0 + j * 512, t0 + (j + 1) * 512)
                        nblk = 4 * j + 4
                        ngrp = nblk // 2
                        u = {h: u_ps.tile([65, 512], F32, tag="u",
                                          name=f"u{h}_{b}_{j}")
                             for h in range(HPC)}

                        def do_av(est, blks):
                            for t2, i in enumerate(blks):
                                for h in range(HPC):
                                    nc.tensor.matmul(
                                        u[h], vph[(h, b)][:, i, 0:65],
                                        est[h][:, t2 * 512:(t2 + 1) * 512],
                                        start=(i == 0), stop=(i == nblk - 1))

                        pend = []
                        for g in range(ngrp):
                            st = {h: st_ps.tile([128, 1024], F32, tag="st",
                                                name=f"st{h}_{b}_{j}_{g}")
                                  for h in range(HPC)}
                            # issue the two heads' matmuls back-to-back:
                            # disjoint 64-row groups run concurrently on PE
                            for t2 in range(2):
                                i = 2 * g + t2
                                ks = slice(t0 + i * 128, t0 + (i + 1) * 128)
                                for h in range(HPC):
                                    hp = h * 64
                                    nc.tensor.matmul(
                                        st[h][:, t2 * 512:(t2 + 1) * 512],
                                        kt[hp:hp + 64, ks],
                                        qt[hp:hp + 64, qs],
                                        start=True, stop=True)
                            est = {}
                            for h in range(HPC):
                                est[h] = estp.tile([128, 1024], F16, tag="est",
                                                   name=f"est{h}_{b}_{j}_{g}")
                                nc.scalar.activation(est[h], st[h], AF.Exp,
                                                     scale=float(DH) ** -0.5,
                                                     bias=ebias)
                                if 2 * g >= 4 * j:  # diagonal group
                                    mo = (2 * g - 4 * j) * 512
                                    nc.vector.tensor_mul(
                                        est[h], est[h], mask[:, mo:mo + 1024])
                            pend.append((est, (2 * g, 2 * g + 1)))
                            if len(pend) > PIPE:
                                do_av(*pend.pop(0))
                            # weave deferred PE work into the exp bubbles
                            if g in (1, 3) and filler:
                                filler.pop(0)()
                        while pend:
                            do_av(*pend.pop(0))

                        # normalize: bcast r (gpsimd), approx 1/r on a
                        # full tile, multiply on DVE
                        for h in range(HPC):
                            hp = h * 64
                            rrow = invp.tile([1, 512], F32, tag="rrow",
                                             name=f"rr{h}_{b}_{j}")
                            nc.vector.tensor_copy(rrow, u[h][64:65, :])
                            bc = invp.tile([64, 512], F32, tag="bc",
                                           name=f"bc{h}_{b}_{j}")
                            nc.gpsimd.partition_broadcast(bc, rrow)
                            bci = invp.tile([64, 512], F32, tag="bci",
                                            name=f"bci{h}_{b}_{j}")
                            nc.vector.reciprocal_approx_fast(bci, bc)
                            nc.vector.tensor_mul(
                                on[hp:hp + 64, j * 512:(j + 1) * 512],
                                u[h][0:64, :], bci)
                        # queue this chunk's out-projection for a later bubble
                        if b == 1 and j == NJ - 1:
                            pass  # tail chunk, emitted at the end
                        else:
                            filler.append(
                                lambda b=b, j=j: outproj_chunk(b, j))

                v_alloc(0)
                for g4 in range(NB // 4):
                    v_transpose_chunk(0, g4)
                v_alloc(1)
                filler.extend(
                    lambda g4=g4: v_transpose_chunk(1, g4)
                    for g4 in range(NB // 4))
                attention(0)
                attention(1)
                while filler:
                    filler.pop(0)()
                outproj_chunk(1, NJ - 1)

    nc.compile()
    return nc


def _get_nc():
    if "nc" not in _CACHE:
        _CACHE["nc"] = _build()
    return _CACHE["nc"]


def _run(nc, in_maps, trace=False):
    from concourse.bass_utils import run_bass_kernel_spmd

    last = None
    for attempt in range(3):
        try:
            return run_bass_kernel_spmd(nc, in_maps,
                                        core_ids=list(range(N_CORES)),
                                        trace=trace)
        except Exception as e:  # transient device faults: retry
            last = e
            if "UNRECOVERABLE" not in str(e) and "UNAVAILABLE" not in str(e):
                raise
    raise last


def kernel(x, w_qkv, w_out, _trace=False):
    x = np.asarray(x, dtype=np.float32)
    w_qkv = np.asarray(w_qkv, dtype=np.float32)
    w_out = np.asarray(w_out, dtype=np.float32)

    # x^T pre-arranged [128, NS, NK, 512]: xt[p, n, k, t] = x^T[k*128+p, n*512+t]
    xt = x.reshape(BT, D).T.astype(np.float16)  # [D, BT]
    xt = np.ascontiguousarray(
        xt.reshape(NK, 128, NS, 512).transpose(1, 2, 0, 3))
    cosb, sinb, mask, ident = _host_consts()

    in_maps = []
    for c in range(N_CORES):
        h0 = HPC * c
        cols = np.arange(h0 * DH, (h0 + HPC) * DH)
        in_maps.append({
            "xt": xt,
            "wq": _prearrange_w(w_qkv[:, cols]),
            "wk": _prearrange_w(w_qkv[:, D + cols]),
            "wv": _prearrange_w(w_qkv[:, 2 * D + cols]),
            "wo": np.ascontiguousarray(w_out[cols, :]
                                       .reshape(128, NK, 128)).astype(np.float16),
            "cosb": cosb,
            "sinb": sinb,
            "mask": mask,
            "ident": ident,
        })

    nc = _get_nc()
    res = _run(nc, in_maps, trace=_trace)
    acc = np.zeros((D, BT), dtype=np.float64)
    for c in range(N_CORES):
        o = res.results[c]["outp"]  # [128, NS, NK, 512]
        acc += o.transpose(2, 0, 1, 3).reshape(D, BT)
    out = acc.T.astype(np.float32).reshape(B, T, D)
    if _trace:
        return out, res
    return out


# revision 9
# speedup vs baseline: 1.4693x; 1.1703x over previous
"""Causal self-attention (B=2, T=2048, D=1024, H=16, rope) on 8 Trainium2 cores.

Sharding: heads are split across cores (2 heads/core, tensor-parallel):
each core computes QKV projection columns for its heads, RoPE, causal
attention, and a partial out-projection (its rows of w_out). The host sums
the 8 partial outputs (the tensor-parallel all-reduce, done at gather time).

All matmul operands are fp16 (fp32 PSUM accumulation). Activations flow
feature-major (transposed) so every matmul contracts along the partition
dim; the host transposes x on the way in and the output back on the way
out. Softmax denominators come free from a ones-column appended to V;
exp runs biased (exp(s/8 - 4)) to stay inside fp16 range, the bias cancels
in the normalization.

Perf structure:
- Host pre-arranges every DRAM tensor so each DMA is one contiguous run
  per partition; x^T is loaded once into per-slice SBUF tiles and stays
  resident. The output uses a permuted [128, NS, NK, 512] layout that the
  host un-permutes. qt/kt/vt are per-batch tiles so nothing waits on a
  whole-tensor dependency.
- The two heads' score matmuls are issued back-to-back: the PE runs them
  concurrently in disjoint 64-row groups (K=64 row-tiling).
- V is transposed on the tensor engine (transpose-mode matmul) in 4-block
  chunks.
- The attention group loop is the spine of the kernel: exp on the scalar
  engine is the pacing resource, so ALL other PE work — QKV projection
  sub-chunks, V-transposes, out-projection chunks — is woven into the
  attention groups through a filler queue. The PE stream never breaks, so
  the PE clock stays un-throttled, and the QKV phase costs no extra
  wall-clock.
"""

import sys

for _p in ("/opt/trn_rl_repo",):
    if _p not in sys.path:
        sys.path.insert(0, _p)

import numpy as np

B, T, D, H = 2, 2048, 1024, 16
DH = D // H  # 64
N_CORES = 8
HPC = H // N_CORES  # heads per core = 2
BT = B * T  # 4096
ROPE_BASE = 10000.0
EXP_BIAS = -4.0

NK = D // 128       # 8 contraction chunks for qkv projection
NS = BT // 512      # 8 token slices
NJ = T // 512       # 4 tq slices per batch
NB = T // 128       # 16 tk blocks per batch
PIPE = 2            # exp->AV software pipeline depth, in 2-block groups

_CACHE = {}


def _host_consts():
    # RoPE tables, feature-major, two heads stacked: [128, T]
    inv_freq = 1.0 / (ROPE_BASE ** (np.arange(0, DH, 2, dtype=np.float32) / DH))
    t = np.arange(T, dtype=np.float32)
    freqs = np.outer(t, inv_freq)  # [T, 32]
    emb = np.concatenate([freqs, freqs], axis=-1)  # [T, 64]
    cosT = np.cos(emb).T.astype(np.float32)  # [64, T]
    sinT = np.sin(emb).T.astype(np.float32)
    # sign baked for the rotate-half term: rows 0:32 get -sin, rows 32:64 +sin
    sinS = np.concatenate([-sinT[:32], sinT[32:]], axis=0)
    cosb = np.concatenate([cosT, cosT], axis=0).astype(np.float16)
    sinb = np.concatenate([sinS, sinS], axis=0).astype(np.float16)
    # Causal masks for the 4 diagonal-block offsets o = 0,128,256,384,
    # concatenated along free dim: [128, 2048]
    p = np.arange(128)[:, None]
    f = np.arange(512)[None, :]
    mask = np.zeros((128, 4 * 512), dtype=np.float16)
    for tno in range(4):
        o = 128 * tno
        mask[:, tno * 512:(tno + 1) * 512] = (f >= o + p).astype(np.float16)
    ident = np.eye(128, dtype=np.float16)
    return cosb, sinb, mask, ident


def _prearrange_w(w):
    # [D, 128] -> [128, NK, 128]: partition-contiguous for one-run DMA
    return np.ascontiguousarray(
        w.reshape(NK, 128, 128).transpose(1, 0, 2)).astype(np.float16)


def _build():
    """Build + schedule the per-core Bass module (same program on all cores)."""
    from concourse import bacc
    import concourse.mybir as mybir
    import concourse.tile as tile

    F16 = mybir.dt.float16
    F32 = mybir.dt.float32
    AF = mybir.ActivationFunctionType

    nc = bacc.Bacc("TRN2", target_bir_lowering=False, debug=False,
                   num_devices=N_CORES)

    xt_d = nc.dram_tensor("xt", [128, NS, NK, 512], F16, kind="ExternalInput")
    wq_d = nc.dram_tensor("wq", [128, NK, 128], F16, kind="ExternalInput")
    wk_d = nc.dram_tensor("wk", [128, NK, 128], F16, kind="ExternalInput")
    wv_d = nc.dram_tensor("wv", [128, NK, 128], F16, kind="ExternalInput")
    wo_d = nc.dram_tensor("wo", [128, NK, 128], F16, kind="ExternalInput")
    cos_d = nc.dram_tensor("cosb", [128, T], F16, kind="ExternalInput")
    sin_d = nc.dram_tensor("sinb", [128, T], F16, kind="ExternalInput")
    mask_d = nc.dram_tensor("mask", [128, 2048], F16, kind="ExternalInput")
    id_d = nc.dram_tensor("ident", [128, 128], F16, kind="ExternalInput")
    out_d = nc.dram_tensor("outp", [128, NS, NK, 512], F16,
                           kind="ExternalOutput")

    with tile.TileContext(nc) as tc:
        with (
            tc.tile_pool(name="consts", bufs=1) as consts,
            tc.tile_pool(name="acts", bufs=1) as acts,
            tc.tile_pool(name="rope", bufs=2) as rope,
            tc.tile_pool(name="vp", bufs=1) as vpp,
            tc.tile_pool(name="est", bufs=6) as estp,
            tc.tile_pool(name="on", bufs=2) as onp,
            tc.tile_pool(name="inv", bufs=2) as invp,
            tc.tile_pool(name="ot", bufs=2) as otp,
            tc.tile_pool(name="st_ps", bufs=3, space="PSUM") as st_ps,
            tc.tile_pool(name="u_ps", bufs=2, space="PSUM") as u_ps,
        ):
            # weights first (gate the first matmuls), x^T per-slice, then
            # the late-use constants on the ACT queue
            wq = consts.tile([128, NK, 128], F16)
            wk = consts.tile([128, NK, 128], F16)
            wv = consts.tile([128, NK, 128], F16)
            xts = [acts.tile([128, NK, 512], F16, name=f"xts{n}")
                   for n in range(NS)]
            nc.sync.dma_start(out=wq, in_=wq_d[:, :, :])
            nc.sync.dma_start(out=xts[0], in_=xt_d[:, 0])
            nc.sync.dma_start(out=wk, in_=wk_d[:, :, :])
            nc.sync.dma_start(out=wv, in_=wv_d[:, :, :])
            for n in range(1, NS):
                nc.sync.dma_start(out=xts[n], in_=xt_d[:, n])
            wo = consts.tile([128, NK, 128], F16)
            cosb = consts.tile([128, T], F16)
            sinb = consts.tile([128, T], F16)
            mask = consts.tile([128, 2048], F16)
            idt = consts.tile([128, 128], F16)
            nc.scalar.dma_start(out=cosb, in_=cos_d[:, :])
            nc.scalar.dma_start(out=sinb, in_=sin_d[:, :])
            nc.scalar.dma_start(out=wo, in_=wo_d[:, :, :])
            nc.scalar.dma_start(out=mask, in_=mask_d[:, :])
            nc.scalar.dma_start(out=idt, in_=id_d[:, :])
            ebias = consts.tile([128, 1], F32)
            nc.vector.memset(ebias, EXP_BIAS)

            # per-batch activations (split so deps stay per-batch)
            qt = {b: acts.tile([128, T], F16, name=f"qt{b}") for b in range(B)}
            kt = {b: acts.tile([128, T], F16, name=f"kt{b}") for b in range(B)}
            vt = {b: acts.tile([128, T], F16, name=f"vt{b}") for b in range(B)}
            vph = {(h, b): vpp.tile([128, NB, 80], F16, name=f"vp{h}_{b}")
                   for h in range(HPC) for b in range(B)}
            for hb in vph:  # V' ones-columns are constant: set them up front
                nc.vector.memset(vph[hb][:, :, 64:65], 1.0)
            ons = {}
            filler = []  # deferred emission chunks, woven into attention

            def qkv_chunk(n, wi):
                """Project one 512-token slice through one of wq/wk/wv."""
                b, jj = n // NJ, n % NJ
                ts = slice(jj * 512, (jj + 1) * 512)
                w = (wq, wk, wv)[wi]
                ps = st_ps.tile([128, 512], F32, tag="st", name=f"ps{n}_{wi}")
                for k in range(NK):
                    nc.tensor.matmul(ps, w[:, k, :], xts[n][:, k, :],
                                     start=(k == 0), stop=(k == NK - 1))
                if wi == 2:
                    nc.scalar.copy(vt[b][:, ts], ps)
                    return
                dst = (qt, kt)[wi][b]
                raw = rope.tile([128, 512], F16, tag="raw", name=f"rw{n}_{wi}")
                nc.scalar.copy(raw, ps)
                swp = rope.tile([128, 512], F16, tag="swp", name=f"sw{n}_{wi}")
                for a, b2 in ((0, 32), (32, 0), (64, 96), (96, 64)):
                    nc.sync.dma_start(out=swp[a:a + 32, :],
                                      in_=raw[b2:b2 + 32, :])
                nc.vector.tensor_mul(raw, raw, cosb[:, ts])
                nc.vector.tensor_mul(swp, swp, sinb[:, ts])
                nc.vector.tensor_add(dst[:, ts], raw, swp)

            def v_transpose_chunk(b, g4):
                """Transpose 4 of batch b's V blocks token-major on PE."""
                tp = st_ps.tile([128, 4, 128], F16, tag="st",
                                name=f"tp_{b}_{g4}")
                for t2 in range(4):
                    i = 4 * g4 + t2
                    blk = slice(i * 128, (i + 1) * 128)
                    nc.tensor.transpose(tp[:, t2, :], vt[b][:, blk], idt)
                for h in range(HPC):
                    src = tp[:, :, h * 64:(h + 1) * 64]
                    dst = vph[(h, b)][:, 4 * g4:4 * g4 + 4, 0:64]
                    if (g4 + h) % 2 == 0:
                        nc.vector.tensor_copy(dst, src)
                    else:
                        nc.scalar.copy(dst, src)

            def outproj_chunk(b, j):
                """Partial out-projection for 512 tokens."""
                onj = ons[b][:, j * 512:(j + 1) * 512]
                ot = otp.tile([128, NK * 512], F16, tag="ot",
                              name=f"ot_{b}_{j}")
                for m2 in range(NK // 2):
                    op = st_ps.tile([128, 1024], F32, tag="st",
                                    name=f"op_{b}_{j}_{m2}")
                    for t2 in range(2):
                        m = 2 * m2 + t2
                        nc.tensor.matmul(
                            op[:, t2 * 512:(t2 + 1) * 512],
                            wo[:, m, :], onj, start=True, stop=True)
                    dst = ot[:, m2 * 1024:(m2 + 1) * 1024]
                    if m2 % 2 == 0:
                        nc.vector.tensor_copy(dst, op)
                    else:
                        nc.scalar.copy(dst, op)
                nc.sync.dma_start(out=out_d[:, b * NJ + j], in_=ot)

            def attention(b):
                on = onp.tile([128, T], F16, tag="on", name=f"on_{b}")
                ons[b] = on
                for j in range(NJ):
                    qs = slice(j * 512, (j + 1) * 512)
                    nblk = 4 * j + 4
                    ngrp = nblk // 2
                    u = {h: u_ps.tile([65, 512], F32, tag="u",
                                      name=f"u{h}_{b}_{j}")
                         for h in range(HPC)}

                    def do_av(est, blks):
                        for t2, i in enumerate(blks):
                            for h in range(HPC):
                                nc.tensor.matmul(
                                    u[h], vph[(h, b)][:, i, 0:65],
                                    est[h][:, t2 * 512:(t2 + 1) * 512],
                                    start=(i == 0), stop=(i == nblk - 1))

                    pend = []
                    for g in range(ngrp):
                        st = {h: st_ps.tile([128, 1024], F32, tag="st",
                                            name=f"st{h}_{b}_{j}_{g}")
                              for h in range(HPC)}
                        # issue the two heads' matmuls back-to-back:
                        # disjoint 64-row groups run concurrently on PE
                        for t2 in range(2):
                            i = 2 * g + t2
                            ks = slice(i * 128, (i + 1) * 128)
                            for h in range(HPC):
                                hp = h * 64
                                nc.tensor.matmul(
                                    st[h][:, t2 * 512:(t2 + 1) * 512],
                                    kt[b][hp:hp + 64, ks],
                                    qt[b][hp:hp + 64, qs],
                                    start=True, stop=True)
                        est = {}
                        for h in range(HPC):
                            est[h] = estp.tile([128, 1024], F16, tag="est",
                                               name=f"est{h}_{b}_{j}_{g}")
                            nc.scalar.activation(est[h], st[h], AF.Exp,
                                                 scale=float(DH) ** -0.5,
                                                 bias=ebias)
                            if 2 * g >= 4 * j:  # diagonal group
                                mo = (2 * g - 4 * j) * 512
                                nc.vector.tensor_mul(
                                    est[h], est[h], mask[:, mo:mo + 1024])
                        pend.append((est, (2 * g, 2 * g + 1)))
                        if len(pend) > PIPE:
                            do_av(*pend.pop(0))
                        # weave deferred PE work into the exp bubbles
                        if g >= 1 and filler:
                            filler.pop(0)()
                    while pend:
                        do_av(*pend.pop(0))

                    # normalize: bcast r (gpsimd), approx 1/r, multiply (DVE)
                    for h in range(HPC):
                        hp = h * 64
                        rrow = invp.tile([1, 512], F32, tag="rrow",
                                         name=f"rr{h}_{b}_{j}")
                        nc.vector.tensor_copy(rrow, u[h][64:65, :])
                        bc = invp.tile([64, 512], F32, tag="bc",
                                       name=f"bc{h}_{b}_{j}")
                        nc.gpsimd.partition_broadcast(bc, rrow)
                        bci = invp.tile([64, 512], F32, tag="bci",
                                        name=f"bci{h}_{b}_{j}")
                        nc.vector.reciprocal_approx_fast(bci, bc)
                        nc.vector.tensor_mul(
                            on[hp:hp + 64, j * 512:(j + 1) * 512],
                            u[h][0:64, :], bci)
                    # queue follow-up work for later bubbles
                    n = b * NJ + j
                    if n + 4 < NS:  # QKV slice n+4 and its V-transpose
                        filler.extend([
                            lambda wi=wi, n=n: qkv_chunk(n + 4, wi)
                            for wi in range(3)])
                        filler.append(
                            lambda n=n: v_transpose_chunk((n + 4) // NJ,
                                                          (n + 4) % NJ))
                    if not (b == 1 and j == NJ - 1):
                        filler.append(lambda b=b, j=j: outproj_chunk(b, j))

            # lead-in: two slices + their transposes emitted directly
            for n in (0, 1):
                for wi in range(3):
                    qkv_chunk(n, wi)
                v_transpose_chunk(0, n)
            filler.extend([lambda wi=wi: qkv_chunk(2, wi) for wi in range(3)])
            filler.append(lambda: v_transpose_chunk(0, 2))
            filler.extend([lambda wi=wi: qkv_chunk(3, wi) for wi in range(3)])
            filler.append(lambda: v_transpose_chunk(0, 3))
            attention(0)
            attention(1)
            while filler:
                filler.pop(0)()
            outproj_chunk(1, NJ - 1)

    nc.compile()
    return nc


def _get_nc():
    if "nc" not in _CACHE:
        _CACHE["nc"] = _build()
    return _CACHE["nc"]


def _run(nc, in_maps, trace=False):
    from concourse.bass_utils import run_bass_kernel_spmd

    last = None
    for attempt in range(3):
        try:
            return run_bass_kernel_spmd(nc, in_maps,
                                        core_ids=list(range(N_CORES)),
                                        trace=trace)
        except Exception as e:  # transient device faults: retry
            last = e
            if "UNRECOVERABLE" not in str(e) and "UNAVAILABLE" not in str(e):
                raise
    raise last


def kernel(x, w_qkv, w_out, _trace=False):
    x = np.asarray(x, dtype=np.float32)
    w_qkv = np.asarray(w_qkv, dtype=np.float32)
    w_out = np.asarray(w_out, dtype=np.float32)

    # x^T pre-arranged [128, NS, NK, 512]: xt[p, n, k, t] = x^T[k*128+p, n*512+t]
    xt = x.reshape(BT, D).T.astype(np.float16)  # [D, BT]
    xt = np.ascontiguousarray(
        xt.reshape(NK, 128, NS, 512).transpose(1, 2, 0, 3))
    cosb, sinb, mask, ident = _host_consts()

    in_maps = []
    for c in range(N_CORES):
        h0 = HPC * c
        cols = np.arange(h0 * DH, (h0 + HPC) * DH)
        in_maps.append({
            "xt": xt,
            "wq": _prearrange_w(w_qkv[:, cols]),
            "wk": _prearrange_w(w_qkv[:, D + cols]),
            "wv": _prearrange_w(w_qkv[:, 2 * D + cols]),
            "wo": np.ascontiguousarray(w_out[cols, :]
                                       .reshape(128, NK, 128)).astype(np.float16),
            "cosb": cosb,
            "sinb": sinb,
            "mask": mask,
            "ident": ident,
        })

    nc = _get_nc()
    res = _run(nc, in_maps, trace=_trace)
    acc = np.zeros((D, BT), dtype=np.float64)
    for c in range(N_CORES):
        o = res.results[c]["outp"]  # [128, NS, NK, 512]
        acc += o.transpose(2, 0, 1, 3).reshape(D, BT)
    out = acc.T.astype(np.float32).reshape(B, T, D)
    if _trace:
        return out, res
    return out


# revision 13
# speedup vs baseline: 1.6247x; 1.1058x over previous
"""Causal self-attention (B=2, T=2048, D=1024, H=16, rope) on 8 Trainium2 cores.

Sharding: heads are split across cores (2 heads/core, tensor-parallel):
each core computes QKV projection columns for its heads, RoPE, causal
attention, and a partial out-projection (its rows of w_out). The host sums
the 8 partial outputs (the tensor-parallel all-reduce, done at gather time).

All matmul operands are fp16 (fp32 PSUM accumulation). Activations flow
feature-major (transposed) so every matmul contracts along the partition
dim; the host transposes x on the way in and the output back on the way
out. Softmax denominators come free from a ones-column appended to V;
exp runs biased (exp(s/8 - 4)) to stay inside fp16 range, the bias cancels
in the normalization.

Perf structure:
- Host pre-arranges every DRAM tensor so each DMA is one contiguous run
  per partition; x^T is loaded once into per-slice SBUF tiles and stays
  resident. The output uses a permuted [128, NS, NK, 512] layout that the
  host un-permutes. qt/kt/vt are per-batch tiles so nothing waits on a
  whole-tensor dependency.
- The two heads' score matmuls are issued back-to-back: the PE runs them
  concurrently in disjoint 64-row groups (K=64 row-tiling).
- V is transposed on the tensor engine (transpose-mode matmul) in 4-block
  chunks.
- The attention group loop is the spine of the kernel: exp on the scalar
  engine is the pacing resource, so ALL other PE work — QKV projection
  sub-chunks, V-transposes, out-projection chunks — is woven into the
  attention groups through a filler queue. The PE stream never breaks, so
  the PE clock stays un-throttled, and the QKV phase costs no extra
  wall-clock.
"""

import sys

for _p in ("/opt/trn_rl_repo",):
    if _p not in sys.path:
        sys.path.insert(0, _p)

import numpy as np

B, T, D, H = 2, 2048, 1024, 16
DH = D // H  # 64
N_CORES = 8
HPC = H // N_CORES  # heads per core = 2
BT = B * T  # 4096
ROPE_BASE = 10000.0
EXP_BIAS = -4.0

NK = D // 128       # 8 contraction chunks for qkv projection
NS = BT // 512      # 8 token slices
NJ = T // 512       # 4 tq slices per batch
NB = T // 128       # 16 tk blocks per batch
PIPE = 2            # exp->AV software pipeline depth, in 2-block groups

_CACHE = {}


# Head-dim row order that makes rotate-half quadrant-local: position p
# holds dim PERM[p]; pairs (d, d+32) land 16 apart inside one 32-row
# quadrant, so the swap is a single DVE stream_shuffle. The score matmul
# contracts over these rows identically for q and k, so the permutation
# cancels there.
PERM = np.concatenate([np.arange(0, 16), np.arange(32, 48),
                       np.arange(16, 32), np.arange(48, 64)])
SHUF_MASK = [(i + 16) % 32 for i in range(32)]


def _host_consts():
    # RoPE tables, feature-major, two heads stacked: [128, T]
    inv_freq = 1.0 / (ROPE_BASE ** (np.arange(0, DH, 2, dtype=np.float32) / DH))
    t = np.arange(T, dtype=np.float32)
    freqs = np.outer(t, inv_freq)  # [T, 32]
    emb = np.concatenate([freqs, freqs], axis=-1)  # [T, 64]
    cosT = np.cos(emb).T.astype(np.float32)[PERM]  # [64, T], row-permuted
    sinT = np.sin(emb).T.astype(np.float32)[PERM]
    # sign baked for the rotate-half term: positions holding dims < 32
    # get -sin (they receive x[d+32]), the rest +sin
    sign = np.where(PERM < 32, -1.0, 1.0)[:, None].astype(np.float32)
    sinS = sinT * sign
    cosb = np.concatenate([cosT, cosT], axis=0).astype(np.float16)
    sinb = np.concatenate([sinS, sinS], axis=0).astype(np.float16)
    # Causal masks for the 4 diagonal-block offsets o = 0,128,256,384,
    # concatenated along free dim: [128, 2048]
    p = np.arange(128)[:, None]
    f = np.arange(512)[None, :]
    mask = np.zeros((128, 4 * 512), dtype=np.float16)
    for tno in range(4):
        o = 128 * tno
        mask[:, tno * 512:(tno + 1) * 512] = (f >= o + p).astype(np.float16)
    ident = np.eye(128, dtype=np.float16)
    return cosb, sinb, mask, ident


def _prearrange_w(w):
    # [D, 128] -> [128, NK, 128]: partition-contiguous for one-run DMA
    return np.ascontiguousarray(
        w.reshape(NK, 128, 128).transpose(1, 0, 2)).astype(np.float16)


def _build():
    """Build + schedule the per-core Bass module (same program on all cores)."""
    from concourse import bacc
    import concourse.mybir as mybir
    import concourse.tile as tile

    F16 = mybir.dt.float16
    F32 = mybir.dt.float32
    AF = mybir.ActivationFunctionType

    nc = bacc.Bacc("TRN2", target_bir_lowering=False, debug=False,
                   num_devices=N_CORES)

    xt_d = nc.dram_tensor("xt", [128, NS, NK, 512], F16, kind="ExternalInput")
    wq_d = nc.dram_tensor("wq", [128, NK, 128], F16, kind="ExternalInput")
    wk_d = nc.dram_tensor("wk", [128, NK, 128], F16, kind="ExternalInput")
    wv_d = nc.dram_tensor("wv", [128, NK, 128], F16, kind="ExternalInput")
    wo_d = nc.dram_tensor("wo", [128, NK, 128], F16, kind="ExternalInput")
    cos_d = nc.dram_tensor("cosb", [128, T], F16, kind="ExternalInput")
    sin_d = nc.dram_tensor("sinb", [128, T], F16, kind="ExternalInput")
    mask_d = nc.dram_tensor("mask", [128, 2048], F16, kind="ExternalInput")
    id_d = nc.dram_tensor("ident", [128, 128], F16, kind="ExternalInput")
    out_d = nc.dram_tensor("outp", [128, NS, NK, 512], F16,
                           kind="ExternalOutput")

    with tile.TileContext(nc) as tc:
        with (
            tc.tile_pool(name="consts", bufs=1) as consts,
            tc.tile_pool(name="acts", bufs=1) as acts,
            tc.tile_pool(name="rope", bufs=2) as rope,
            tc.tile_pool(name="vp", bufs=1) as vpp,
            tc.tile_pool(name="est", bufs=6) as estp,
            tc.tile_pool(name="on", bufs=2) as onp,
            tc.tile_pool(name="inv", bufs=2) as invp,
            tc.tile_pool(name="ot", bufs=2) as otp,
            tc.tile_pool(name="st_ps", bufs=3, space="PSUM") as st_ps,
            tc.tile_pool(name="u_ps", bufs=2, space="PSUM") as u_ps,
        ):
            # weights first (gate the first matmuls), x^T per-slice, then
            # the late-use constants on the ACT queue
            wq = consts.tile([128, NK, 128], F16)
            wk = consts.tile([128, NK, 128], F16)
            wv = consts.tile([128, NK, 128], F16)
            xts = [acts.tile([128, NK, 512], F16, name=f"xts{n}")
                   for n in range(NS)]
            nc.sync.dma_start(out=wq, in_=wq_d[:, :, :])
            nc.sync.dma_start(out=xts[0], in_=xt_d[:, 0])
            nc.sync.dma_start(out=wk, in_=wk_d[:, :, :])
            nc.sync.dma_start(out=wv, in_=wv_d[:, :, :])
            for n in range(1, NS):
                nc.sync.dma_start(out=xts[n], in_=xt_d[:, n])
            wo = consts.tile([128, NK, 128], F16)
            cosb = consts.tile([128, T], F16)
            sinb = consts.tile([128, T], F16)
            mask = consts.tile([128, 2048], F16)
            idt = consts.tile([128, 128], F16)
            nc.scalar.dma_start(out=cosb, in_=cos_d[:, :])
            nc.scalar.dma_start(out=sinb, in_=sin_d[:, :])
            nc.scalar.dma_start(out=wo, in_=wo_d[:, :, :])
            nc.scalar.dma_start(out=mask, in_=mask_d[:, :])
            nc.scalar.dma_start(out=idt, in_=id_d[:, :])
            ebias = consts.tile([128, 1], F32)
            nc.vector.memset(ebias, EXP_BIAS)

            # per-batch activations (split so deps stay per-batch)
            qt = {b: acts.tile([128, T], F16, name=f"qt{b}") for b in range(B)}
            kt = {b: acts.tile([128, T], F16, name=f"kt{b}") for b in range(B)}
            vt = {b: acts.tile([128, T], F16, name=f"vt{b}") for b in range(B)}
            vph = {(h, b): vpp.tile([128, NB, 80], F16, name=f"vp{h}_{b}")
                   for h in range(HPC) for b in range(B)}
            for hb in vph:  # V' ones-columns are constant: set them up front
                nc.vector.memset(vph[hb][:, :, 64:65], 1.0)
            ons = {}
            filler = []  # deferred emission chunks, woven into attention

            def qkv_chunk(n, wi):
                """Project one 512-token slice through one of wq/wk/wv."""
                b, jj = n // NJ, n % NJ
                ts = slice(jj * 512, (jj + 1) * 512)
                w = (wq, wk, wv)[wi]
                ps = st_ps.tile([128, 512], F32, tag="st", name=f"ps{n}_{wi}")
                for k in range(NK):
                    nc.tensor.matmul(ps, w[:, k, :], xts[n][:, k, :],
                                     start=(k == 0), stop=(k == NK - 1))
                if wi == 2:
                    nc.scalar.copy(vt[b][:, ts], ps)
                    return
                dst = (qt, kt)[wi][b]
                raw = rope.tile([128, 512], F16, tag="raw", name=f"rw{n}_{wi}")
                nc.scalar.copy(raw, ps)
                # rotate-half is quadrant-local under PERM: one shuffle
                swp = rope.tile([128, 512], F16, tag="swp", name=f"sw{n}_{wi}")
                nc.vector.stream_shuffle(swp, raw, SHUF_MASK)
                nc.vector.tensor_mul(raw, raw, cosb[:, ts])
                nc.vector.tensor_mul(swp, swp, sinb[:, ts])
                nc.vector.tensor_add(dst[:, ts], raw, swp)

            def v_transpose_chunk(b, g4):
                """Transpose 4 of batch b's V blocks token-major on PE."""
                tp = st_ps.tile([128, 4, 128], F16, tag="st",
                                name=f"tp_{b}_{g4}")
                for t2 in range(4):
                    i = 4 * g4 + t2
                    blk = slice(i * 128, (i + 1) * 128)
                    nc.tensor.transpose(tp[:, t2, :], vt[b][:, blk], idt)
                for h in range(HPC):
                    src = tp[:, :, h * 64:(h + 1) * 64]
                    dst = vph[(h, b)][:, 4 * g4:4 * g4 + 4, 0:64]
                    if (g4 + h) % 2 == 0:
                        nc.vector.tensor_copy(dst, src)
                    else:
                        nc.scalar.copy(dst, src)

            def outproj_chunk(b, j):
                """Partial out-projection for 512 tokens."""
                onj = ons[b][:, j * 512:(j + 1) * 512]
                ot = otp.tile([128, NK * 512], F16, tag="ot",
                              name=f"ot_{b}_{j}")
                for m2 in range(NK // 2):
                    op = st_ps.tile([128, 1024], F32, tag="st",
                                    name=f"op_{b}_{j}_{m2}")
                    for t2 in range(2):
                        m = 2 * m2 + t2
                        nc.tensor.matmul(
                            op[:, t2 * 512:(t2 + 1) * 512],
                            wo[:, m, :], onj, start=True, stop=True)
                    dst = ot[:, m2 * 1024:(m2 + 1) * 1024]
                    if m2 % 2 == 0:
                        nc.vector.tensor_copy(dst, op)
                    else:
                        nc.scalar.copy(dst, op)
                    if m2 % 2 == 1:  # ship each half as soon as it's staged
                        nc.sync.dma_start(
                            out=out_d[:, b * NJ + j, 2 * (m2 - 1):2 * (m2 + 1)],
                            in_=ot[:, (m2 - 1) * 1024:(m2 + 1) * 1024])

            def attention(b):
                on = onp.tile([128, T], F16, tag="on", name=f"on_{b}")
                ons[b] = on
                for j in range(NJ):
                    qs = slice(j * 512, (j + 1) * 512)
                    nblk = 4 * j + 4
                    ngrp = nblk // 2
                    u = {h: u_ps.tile([65, 512], F32, tag="u",
                                      name=f"u{h}_{b}_{j}")
                         for h in range(HPC)}

                    def do_av(est, blks):
                        for t2, i in enumerate(blks):
                            for h in range(HPC):
                                nc.tensor.matmul(
                                    u[h], vph[(h, b)][:, i, 0:65],
                                    est[h][:, t2 * 512:(t2 + 1) * 512],
                                    start=(i == 0), stop=(i == nblk - 1))

                    pend = []
                    for g in range(ngrp):
                        st = {h: st_ps.tile([128, 1024], F32, tag="st",
                                            name=f"st{h}_{b}_{j}_{g}")
                              for h in range(HPC)}
                        # issue the two heads' matmuls back-to-back:
                        # disjoint 64-row groups run concurrently on PE
                        for t2 in range(2):
                            i = 2 * g + t2
                            ks = slice(i * 128, (i + 1) * 128)
                            for h in range(HPC):
                                hp = h * 64
                                nc.tensor.matmul(
                                    st[h][:, t2 * 512:(t2 + 1) * 512],
                                    kt[b][hp:hp + 64, ks],
                                    qt[b][hp:hp + 64, qs],
                                    start=True, stop=True)
                        est = {}
                        for h in range(HPC):
                            est[h] = estp.tile([128, 1024], F16, tag="est",
                                               name=f"est{h}_{b}_{j}_{g}")
                            nc.scalar.activation(est[h], st[h], AF.Exp,
                                                 scale=float(DH) ** -0.5,
                                                 bias=ebias)
                            if 2 * g >= 4 * j:  # diagonal group
                                mo = (2 * g - 4 * j) * 512
                                nc.vector.tensor_mul(
                                    est[h], est[h], mask[:, mo:mo + 1024])
                        pend.append((est, (2 * g, 2 * g + 1)))
                        if len(pend) > PIPE:
                            do_av(*pend.pop(0))
                        # weave deferred PE work into the exp bubbles
                        if g >= 1 and filler:
                            filler.pop(0)()
                    while pend:
                        do_av(*pend.pop(0))

                    # normalize: bcast r (gpsimd), approx 1/r, multiply (DVE)
                    for h in range(HPC):
                        hp = h * 64
                        rrow = invp.tile([1, 512], F32, tag="rrow",
                                         name=f"rr{h}_{b}_{j}")
                        nc.vector.tensor_copy(rrow, u[h][64:65, :])
                        bc = invp.tile([64, 512], F32, tag="bc",
                                       name=f"bc{h}_{b}_{j}")
                        nc.gpsimd.partition_broadcast(bc, rrow)
                        bci = invp.tile([64, 512], F32, tag="bci",
                                        name=f"bci{h}_{b}_{j}")
                        nc.vector.reciprocal_approx_fast(bci, bc)
                        nc.vector.tensor_mul(
                            on[hp:hp + 64, j * 512:(j + 1) * 512],
                            u[h][0:64, :], bci)
                    # queue follow-up work for later bubbles
                    n = b * NJ + j
                    if n + 4 < NS:  # QKV slice n+4 and its V-transpose
                        filler.extend([
                            lambda wi=wi, n=n: qkv_chunk(n + 4, wi)
                            for wi in range(3)])
                        filler.append(
                            lambda n=n: v_transpose_chunk((n + 4) // NJ,
                                                          (n + 4) % NJ))
                    if not (b == 1 and j == NJ - 1):
                        filler.append(lambda b=b, j=j: outproj_chunk(b, j))

            # lead-in: two slices + their transposes emitted directly
            for n in (0, 1):
                for wi in range(3):
                    qkv_chunk(n, wi)
                v_transpose_chunk(0, n)
            filler.extend([lambda wi=wi: qkv_chunk(2, wi) for wi in range(3)])
            filler.append(lambda: v_transpose_chunk(0, 2))
            filler.extend([lambda wi=wi: qkv_chunk(3, wi) for wi in range(3)])
            filler.append(lambda: v_transpose_chunk(0, 3))
            attention(0)
            attention(1)
            while filler:
                filler.pop(0)()
            outproj_chunk(1, NJ - 1)

    nc.compile()
    return nc


def _get_nc():
    if "nc" not in _CACHE:
        _CACHE["nc"] = _build()
    return _CACHE["nc"]


def _run(nc, in_maps, trace=False):
    from concourse.bass_utils import run_bass_kernel_spmd

    last = None
    for attempt in range(3):
        try:
            return run_bass_kernel_spmd(nc, in_maps,
                                        core_ids=list(range(N_CORES)),
                                        trace=trace)
        except Exception as e:  # transient device faults: retry
            last = e
            if "UNRECOVERABLE" not in str(e) and "UNAVAILABLE" not in str(e):
                raise
    raise last


def kernel(x, w_qkv, w_out, _trace=False):
    x = np.asarray(x, dtype=np.float32)
    w_qkv = np.asarray(w_qkv, dtype=np.float32)
    w_out = np.asarray(w_out, dtype=np.float32)

    # x^T pre-arranged [128, NS, NK, 512]: xt[p, n, k, t] = x^T[k*128+p, n*512+t]
    xt = x.reshape(BT, D).T.astype(np.float16)  # [D, BT]
    xt = np.ascontiguousarray(
        xt.reshape(NK, 128, NS, 512).transpose(1, 2, 0, 3))
    cosb, sinb, mask, ident = _host_consts()

    in_maps = []
    for c in range(N_CORES):
        h0 = HPC * c
        cols = np.arange(h0 * DH, (h0 + HPC) * DH)
        # q/k columns in PERM order (quadrant-local rotate-half)
        pcols = np.concatenate([(h0 + h) * DH + PERM for h in range(HPC)])
        in_maps.append({
            "xt": xt,
            "wq": _prearrange_w(w_qkv[:, pcols]),
            "wk": _prearrange_w(w_qkv[:, D + pcols]),
            "wv": _prearrange_w(w_qkv[:, 2 * D + cols]),
            "wo": np.ascontiguousarray(w_out[cols, :]
                                       .reshape(128, NK, 128)).astype(np.float16),
            "cosb": cosb,
            "sinb": sinb,
            "mask": mask,
            "ident": ident,
        })

    nc = _get_nc()
    res = _run(nc, in_maps, trace=_trace)
    acc = np.zeros((D, BT), dtype=np.float64)
    for c in range(N_CORES):
        o = res.results[c]["outp"]  # [128, NS, NK, 512]
        acc += o.transpose(2, 0, 1, 3).reshape(D, BT)
    out = acc.T.astype(np.float32).reshape(B, T, D)
    if _trace:
        return out, res
    return out
